# revision 7
# baseline (speedup 1.0000x reference)
"""GAT (2-layer) Trainium2 Bass kernel — 8-core SPMD, v2.

Sharding: dst nodes across 8 cores (12500 each), packed into 98 windows of
128 (one SBUF partition per dst), profile-sorted. Edge rows (256B: h only)
are fetched by gpsimd.dma_gather from node tables (4 src-groups of 25088
rows for int16 indices). v2 vs v1:
  - batch-uniform slot layout (W consecutive windows share per-group width
    Ug) so attention + weighted segment-sum run as ~40 large 4D-AP DVE ops
    per batch instead of ~140 small per-window ops;
  - 256B rows (h only); layer-1 a_s is recomputed on the fly from gathered
    h (mult+reduce vs replicated att1_s); pad slots contribute
    exp(lrelu(a_d)) to the softmax denominator, subtracted exactly via a
    host-computed per-(dst,window) pad count;
  - layer-2 rows carry a_s2 in bf16 at col 64 (pad row poisoned to -300);
  - gather DMA waits moved out of the prep critical section (prep b+1
    overlaps DMA b); one idx load + one trigger per batch;
  - phase 0 sharded: each core computes its 12544-row slice of the dense
    h1 table, AllGathered while the per-window a_d pass runs;
  - layer-2 log-softmax ln() deferred to one end pass; single output DMA.
"""

import os
import numpy as np
import ml_dtypes

import concourse.bacc as bacc
import concourse.bass as bass
import concourse.mybir as mybir
import concourse.tile as tile
from concourse.bass_utils import run_bass_kernel_spmd
from concourse.masks import make_identity

F32 = mybir.dt.float32
BF16 = mybir.dt.bfloat16
I16 = mybir.dt.int16
AX = mybir.AxisListType
OP = mybir.AluOpType
ACT = mybir.ActivationFunctionType

N, E = 100000, 1600000
IN, HID, OUT, HEADS = 256, 16, 64, 8
NEG = 0.2
NCORES = 8
NSH = N // NCORES        # 12500
NGRP = 4
GSZ = N // NGRP          # 25000
NP = 25088               # table rows per group
NW = 98                  # windows per core
SH_ROWS = NW * 128       # 12544 = rows per core shard of both tables
PAD1 = NSH               # group-local pad row, table1 (all-zero row)
PAD2 = NSH               # group-local pad row, table2 (a_s2 = -300)
ROW1 = 256               # bf16 elems per table1 row (512B: a_s f32x8 | h 128)
ROW2 = 128               # bf16 elems per table2 row (256B: h2 64 | a_s2 | pad)
COLS1 = 96               # padded slot columns per layer-1 batch
COLS2 = 120              # padded slot columns per layer-2 batch
# per-batch gather descs = cols*128 must stay under the 16384-desc SWDGE ring
STAGE_ELEMS = COLS1 * ROW1   # >= COLS2 * ROW2, shared stag tile size
WMAX = 7                 # max windows per batch (W*66 <= 512 psum floats)
A_NEG = -300.0


# ---------------------------------------------------------------- host side
def _layout(src, dst):
    core = dst // NSH
    grp = src // GSZ
    cg_all = np.zeros((NCORES, NSH, NGRP), np.int32)
    np.add.at(cg_all, (core, dst % NSH, grp), 1)
    perms = []
    for k in range(NCORES):
        cg = cg_all[k]
        perms.append(np.lexsort((cg[:, 3], cg[:, 2], cg[:, 1], cg[:, 0]))[::-1])
    Lg = np.zeros((NW, NGRP), np.int64)
    for k in range(NCORES):
        cgp = cg_all[k][perms[k]]
        cgp = np.concatenate([cgp, np.zeros((SH_ROWS - NSH, NGRP), np.int32)])
        Lg = np.maximum(Lg, cgp.reshape(NW, 128, NGRP).max(axis=1))
    sig = np.empty(N, np.int64)
    for k in range(NCORES):
        pos = np.empty(NSH, np.int64)
        pos[perms[k]] = np.arange(NSH)
        sig[k * NSH:(k + 1) * NSH] = k * SH_ROWS + pos
    eorder = np.lexsort((grp, dst))
    es, ed, eg, ec = src[eorder], dst[eorder], grp[eorder], core[eorder]
    core_starts = np.searchsorted(ec, np.arange(NCORES + 1))
    cores = [(es[a:b], (ed[a:b] - k * NSH), eg[a:b])
             for k, (a, b) in enumerate(zip(core_starts[:-1], core_starts[1:]))]
    return dict(Lg=Lg, perms=perms, sig=sig, cores=cores)


def _make_batches(Lg, budget):
    batches = []
    w = 0
    while w < NW:
        best = None
        for W in range(1, WMAX + 1):
            if w + W > NW:
                break
            Ug = Lg[w:w + W].max(axis=0)
            cols = W * int(Ug.sum())
            if cols <= budget:
                best = (W, Ug)
        if best is None:
            best = (1, Lg[w:w + 1].max(axis=0))
        W, Ug = best
        Ug = [int(u) for u in Ug]
        batches.append(dict(w0=w, W=W, Ug=Ug, Utot=sum(Ug)))
        w += W
    return batches


def _pack_idx(arr_pj):
    """[128, cols] slot-array of indices -> wrapped idx rows [16, cols*8]."""
    I = arr_pj.T.ravel()                      # I[j*128+p]
    return I.reshape(-1, 16).T.astype(np.int16)  # [16, len/16]


def _host_inputs(inputs, lay, batches1, batches2):
    x = np.asarray(inputs["x"], np.float32)
    W1 = np.asarray(inputs["W1"], np.float64)
    att1_s = np.asarray(inputs["att1_s"], np.float64)
    att1_d = np.asarray(inputs["att1_d"], np.float64)
    W2 = np.asarray(inputs["W2"], np.float64)
    att2_s = np.asarray(inputs["att2_s"], np.float64)
    att2_d = np.asarray(inputs["att2_d"], np.float64)
    b1 = np.asarray(inputs["b1"], np.float32)
    b2 = np.asarray(inputs["b2"], np.float32)
    Lg, perms, sig = lay["Lg"], lay["perms"], lay["sig"]

    A_s = np.zeros((HEADS * HID, HEADS))
    A_d = np.zeros((HEADS * HID, HEADS))
    for h in range(HEADS):
        A_s[h * HID:(h + 1) * HID, h] = att1_s[h]
        A_d[h * HID:(h + 1) * HID, h] = att1_d[h]
    # c-major head channels (e' = c*8+h) so the layer-1 message multiply has
    # a unit-stride innermost head axis on every operand (DVE 16-bit 2x path)
    cperm = np.arange(HEADS * HID).reshape(HEADS, HID).T.ravel()
    w1r = np.concatenate([W1[:, cperm], W1 @ A_s, W1 @ A_d], axis=1)  # [256,144]
    w2r = np.concatenate([W2, W2 @ att2_s.T, W2 @ att2_d.T],
                         axis=1)[cperm, :]                            # [128,66]
    w1r_bf = np.ascontiguousarray(w1r.astype(ml_dtypes.bfloat16))
    w2r_bf = np.ascontiguousarray(w2r.astype(ml_dtypes.bfloat16))

    cst_base = np.zeros((128, 290), np.float32)
    cst_base[:, 0:128] = b1[cperm][None, :]
    cst_base[:, 128:192] = b2[None, :]

    per_core = []
    for k in range(NCORES):
        es, edl, eg = lay["cores"][k]
        pos = np.empty(NSH, np.int64)
        pos[perms[k]] = np.arange(NSH)
        o = np.lexsort((eg, pos[edl]))
        es_o, eg_o, pos_o = es[o], eg[o], pos[edl][o]
        w_o, p_o = pos_o // 128, pos_o % 128
        key = pos_o * NGRP + eg_o
        slot = np.arange(len(o)) - np.searchsorted(key, key)
        deg = np.zeros((128, NW), np.int32)
        np.add.at(deg, (p_o, w_o), 1)
        npad = np.zeros((128, NW), np.float32)
        secs = []
        sigl = sig[es_o] % NP
        for li, batches in enumerate((batches1, batches2)):
            vals = sigl
            padv = PAD1 if li == 0 else PAD2
            for b in batches:
                w0, W, Ug = b["w0"], b["W"], b["Ug"]
                if li == 0:
                    npad[:, w0:w0 + W] = b["Utot"] - deg[:, w0:w0 + W]
                inb = (w_o >= w0) & (w_o < w0 + W)
                for g in range(NGRP):
                    if Ug[g] == 0:
                        continue
                    a = np.full((128, W * Ug[g]), padv, np.int64)
                    m = inb & (eg_o == g)
                    a[p_o[m], (w_o[m] - w0) * Ug[g] + slot[m]] = vals[m]
                    secs.append(a)
        idx_blob = np.concatenate([_pack_idx(a) for a in secs], axis=1)
        xtp = np.zeros((IN, SH_ROWS), np.float32)
        xtp[:, :NSH] = x[k * NSH:(k + 1) * NSH].T[:, perms[k]]
        cst = cst_base.copy()
        cst[:, 192:290] = npad
        per_core.append({
            "xtp": np.ascontiguousarray(xtp.astype(ml_dtypes.bfloat16)),
            "w1r": w1r_bf,
            "w2r": w2r_bf,
            "cst": np.ascontiguousarray(cst),
            "idx": np.ascontiguousarray(idx_blob),
        })
    return per_core


# ------------------------------------------------------------- device side
def _build_program(Lg, batches1, batches2):
    nc = bacc.Bacc("TRN2", target_bir_lowering=False, debug=False,
                   num_devices=NCORES)
    IDXF = 8 * (sum(b["W"] * b["Utot"] for b in batches1)
                + sum(b["W"] * b["Utot"] for b in batches2))
    MAXGRP = max(max(b["W"] * max(b["Ug"]) for b in batches1) * HEADS * HID,
                 max(b["W"] * max(b["Ug"]) for b in batches2) * OUT)
    xtp = nc.declare_dram_parameter("xtp", [256, SH_ROWS], BF16, isOutput=False)
    w1r = nc.declare_dram_parameter("w1r", [256, 144], BF16, isOutput=False)
    w2r = nc.declare_dram_parameter("w2r", [128, 66], BF16, isOutput=False)
    cst = nc.declare_dram_parameter("cst", [128, 290], F32, isOutput=False)
    idxp = nc.declare_dram_parameter("idx", [16, IDXF], I16, isOutput=False)
    outp = nc.declare_dram_parameter("out", [SH_ROWS, OUT], F32, isOutput=True)

    shard1 = nc.dram_tensor("shard1", [SH_ROWS, ROW1], BF16)
    table1 = nc.dram_tensor("table1", [NCORES * SH_ROWS, ROW1], BF16)
    shard2 = nc.dram_tensor("shard2", [SH_ROWS, ROW2], BF16)
    table2 = nc.dram_tensor("table2", [NCORES * SH_ROWS, ROW2], BF16)

    dma_sem = nc.alloc_semaphore("g_dma")
    prep_sem = nc.alloc_semaphore("g_prep")
    cc_sem = nc.alloc_semaphore("cc")
    gn = [0]   # gathers issued
    cn = [0]   # collectives issued

    CH = 14          # windows/tiles per phase-0 chunk
    NCH = NW // CH   # 7

    with tile.TileContext(nc) as tc:
        with (
            tc.tile_pool(name="const", bufs=1) as constp,
            tc.tile_pool(name="psum", bufs=2, space="PSUM") as psump,
        ):
            w1r0_t = constp.tile([128, 144], BF16, tag="w1r0")
            w1r1_t = constp.tile([128, 144], BF16, tag="w1r1")
            w2r_t = constp.tile([128, 66], BF16, tag="w2r")
            cst_t = constp.tile([128, 290], F32, tag="cst")
            ident = constp.tile([128, 128], BF16, tag="ident")
            adwin = constp.tile([128, NW * HEADS], F32, tag="adwin")
            ad2win = constp.tile([128, NW], F32, tag="ad2win")
            npe1 = constp.tile([128, NW * HEADS], F32, tag="npe1")
            shbuf = constp.tile([128, NW * OUT], F32, tag="shbuf")
            sebuf = constp.tile([128, NW], F32, tag="sebuf")
            nc.sync.dma_start(out=w1r0_t[:], in_=w1r[0:128, :])
            nc.sync.dma_start(out=w1r1_t[:], in_=w1r[128:256, :])
            nc.sync.dma_start(out=w2r_t[:], in_=w2r[:])
            nc.sync.dma_start(out=cst_t[:], in_=cst[:])
            make_identity(nc, ident[:])
            b1v = cst_t[:, 0:128]
            b2v = cst_t[:, 128:192]
            npadv = cst_t[:, 192:290]

            # ---------------- phase 0: own slice of dense h1 table ---------
            with (
                tc.tile_pool(name="xt", bufs=2) as xtpool,
                tc.tile_pool(name="dense", bufs=2) as densep,
            ):
                # single fused pass: h1 rows (perm order, = table2 order),
                # in-row a_s, and per-window a_d, all from xtp
                for ch in range(NCH):
                    base = ch * CH * 128
                    xs0 = xtpool.tile([128, CH * 128], BF16, tag="xs0")
                    xs1 = xtpool.tile([128, CH * 128], BF16, tag="xs1")
                    nc.sync.dma_start(out=xs0[:], in_=xtp[0:128, base:base + CH * 128])
                    nc.sync.dma_start(out=xs1[:], in_=xtp[128:256, base:base + CH * 128])
                    rows = densep.tile([128, CH * 144], BF16, tag="rows")
                    for t in range(CH):
                        ps = psump.tile([128, 144], F32, tag="ps0")
                        nc.tensor.matmul(
                            out=ps[:], lhsT=xs0[:, t * 128:(t + 1) * 128],
                            rhs=w1r0_t[:, 0:144], start=True, stop=False)
                        nc.tensor.matmul(
                            out=ps[:], lhsT=xs1[:, t * 128:(t + 1) * 128],
                            rhs=w1r1_t[:, 0:144], start=False, stop=True)
                        rv = rows[:, t * 144:(t + 1) * 144]
                        nc.scalar.activation(rv[0:128, 0:16].bitcast(F32),
                                             ps[:, 128:136], ACT.Copy, 0.0, 1.0)
                        nc.scalar.activation(rv[0:128, 16:144],
                                             ps[:, 0:128], ACT.Copy, 0.0, 1.0)
                        nc.vector.tensor_copy(
                            out=adwin[:, (ch * CH + t) * 8:(ch * CH + t + 1) * 8],
                            in_=ps[:, 136:144])
                    nc.sync.dma_start(
                        out=shard1[base:base + CH * 128, 0:144]
                            .rearrange("(a p) r -> p a r", p=128),
                        in_=rows[:].rearrange("p (a r) -> p a r", a=CH))

                with tc.tile_critical():
                    nc.gpsimd.collective_compute(
                        "AllGather", OP.bypass,
                        replica_groups=[list(range(NCORES))],
                        ins=[shard1[:]], outs=[table1[:]],
                    ).then_inc(cc_sem, 1)
                    nc.gpsimd.wait_ge(cc_sem, 1)
                cn[0] += 1

            # npe1 = npad * exp(lrelu(adwin))
            nc.vector.tensor_scalar_mul(npe1[:], adwin[:], NEG)
            nc.vector.tensor_tensor(out=npe1[:], in0=adwin[:], in1=npe1[:],
                                    op=OP.max)
            nc.scalar.activation(npe1[:], npe1[:], ACT.Exp, 0.0, 1.0)
            nc.vector.tensor_tensor(
                out=npe1[:].rearrange("p (w h) -> p w h", w=NW),
                in0=npe1[:].rearrange("p (w h) -> p w h", w=NW),
                in1=npadv.rearrange("p (w h) -> p w h", h=1)
                    .to_broadcast([128, NW, HEADS]),
                op=OP.mult)

            # ---------------- edge layers ----------------------------------
            ctx_edge = __import__("contextlib").ExitStack()
            stagp = ctx_edge.enter_context(tc.tile_pool(name="stag", bufs=2))
            idxpool = ctx_edge.enter_context(tc.tile_pool(name="idx", bufs=2))
            workp = ctx_edge.enter_context(tc.tile_pool(name="work", bufs=2))
            scrp = ctx_edge.enter_context(tc.tile_pool(name="scr", bufs=1))
            smallp = ctx_edge.enter_context(tc.tile_pool(name="small", bufs=2))
            idx_off = [0]

            def issue_gathers(layer, b, stag, ixt):
                tabl, row = (table1, ROW1) if layer == 1 else (table2, ROW2)
                W, Ug = b["W"], b["Ug"]
                ng0 = gn[0]
                with tc.tile_critical(no_gpsimd_drain=True):
                    goff = 0
                    for g in range(NGRP):
                        if Ug[g] == 0:
                            continue
                        Kg = W * Ug[g]
                        sl3 = stag[:, goff * row:(goff + Kg) * row] \
                            .rearrange("p (k r) -> p k r", r=row)
                        gn[0] += 1
                        nc.gpsimd.dma_gather(
                            out_ap=sl3, in_ap=tabl[g * NP:(g + 1) * NP, :],
                            idxs_ap=ixt[:, goff * 8:(goff + Kg) * 8],
                            num_idxs=128 * Kg, num_idxs_reg=128 * Kg,
                            elem_size=row, single_packet=False,
                            prepare_only=True, sem=dma_sem,
                        ).then_inc(prep_sem, 1)
                        goff += Kg
                    nc.gpsimd.wait_ge(prep_sem, gn[0])
                    nc.gpsimd.trigger_dma(count=gn[0] - ng0)
                return gn[0]

            def await_gather(layer, b, stag, gtarget):
                row = ROW1 if layer == 1 else ROW2
                cols = b["W"] * b["Utot"]
                with tc.tile_critical(no_gpsimd_drain=True):
                    nc.gpsimd.wait_ge(dma_sem, 16 * gtarget)
                    v = stag[:, 0:cols * row].rearrange("p (k r) -> p k r", r=row)
                    nc.gpsimd.tensor_copy(out=v[:, :, 0:1], in_=v[:, :, 0:1])

            def load_batch(layer, b):
                cols = b["W"] * b["Utot"]
                ixt = idxpool.tile([128, COLS2 * 8], I16, tag="ix")
                for rep in range(8):
                    nc.sync.dma_start(
                        out=ixt[rep * 16:(rep + 1) * 16, 0:cols * 8],
                        in_=idxp[:, idx_off[0]:idx_off[0] + cols * 8])
                idx_off[0] += cols * 8
                stag = stagp.tile([128, STAGE_ELEMS], BF16, tag="st")
                gtarget = issue_gathers(layer, b, stag, ixt)
                return stag, gtarget

            def compute_batch(layer, b, stag):
                w0, W, Ug, Utot = b["w0"], b["W"], b["Ug"], b["Utot"]
                nh = HEADS if layer == 1 else 1
                nch = HID if layer == 1 else OUT
                row = ROW1 if layer == 1 else ROW2
                hoff = 16 if layer == 1 else 0
                wall_t = workp.tile([128, COLS1 * HEADS], F32, tag="wa")
                # --- logits = a_s (in-row) + a_d, group-major into wall -----
                goff = 0
                for g in range(NGRP):
                    if Ug[g] == 0:
                        continue
                    Kg = W * Ug[g]
                    sec4 = stag[:, goff * row:(goff + Kg) * row].rearrange(
                        "p (w l r) -> p w l r", w=W, r=row)
                    if layer == 1:
                        asv = sec4[:, :, :, 0:16].bitcast(F32)
                        adv = adwin[:, w0 * 8:(w0 + W) * 8]
                    else:
                        asv = sec4[:, :, :, 64:65]
                        adv = ad2win[:, w0:w0 + W]
                    wv = wall_t[:, goff * nh:(goff + Kg) * nh]
                    nc.vector.tensor_tensor(
                        out=wv.rearrange("p (w l h) -> p w l h", w=W, h=nh),
                        in0=asv,
                        in1=adv.rearrange("p (w l h) -> p w l h", w=W, l=1)
                            .to_broadcast([128, W, Ug[g], nh]),
                        op=OP.add)
                    goff += Kg
                wall = wall_t[:, 0:Utot * W * nh]
                # --- lrelu + exp --------------------------------------------
                lr = smallp.tile([128, COLS1 * HEADS], F32, tag="lr")
                nc.vector.tensor_scalar_mul(lr[:, 0:Utot * W * nh], wall, NEG)
                nc.vector.tensor_tensor(out=wall, in0=wall,
                                        in1=lr[:, 0:Utot * W * nh], op=OP.max)
                nc.scalar.activation(wall, wall, ACT.Exp, 0.0, 1.0)
                # --- denominator -------------------------------------------
                den = smallp.tile([128, WMAX * HEADS], F32, tag="den")
                dent = smallp.tile([128, WMAX * HEADS], F32, tag="dent")
                goff = 0
                first = True
                for g in range(NGRP):
                    if Ug[g] == 0:
                        continue
                    Kg = W * Ug[g]
                    tgt = den if first else dent
                    nc.vector.tensor_reduce(
                        out=tgt[:, 0:W * nh].rearrange("p (w h) -> p w h", h=nh),
                        in_=wall_t[:, goff * nh:(goff + Kg) * nh].rearrange(
                            "p (w l h) -> p w h l", w=W, h=nh),
                        axis=AX.X, op=OP.add)
                    if not first:
                        nc.vector.tensor_tensor(out=den[:, 0:W * nh],
                                                in0=den[:, 0:W * nh],
                                                in1=dent[:, 0:W * nh], op=OP.add)
                    first = False
                    goff += Kg
                if layer == 1:
                    nc.vector.tensor_tensor(out=den[:, 0:W * nh],
                                            in0=den[:, 0:W * nh],
                                            in1=npe1[:, w0 * 8:(w0 + W) * 8],
                                            op=OP.subtract)
                nc.vector.tensor_scalar_max(den[:, 0:W * nh], den[:, 0:W * nh],
                                            1e-30)
                rec = smallp.tile([128, WMAX * HEADS], F32, tag="rec")
                nc.vector.reciprocal(rec[:, 0:W * nh], den[:, 0:W * nh])
                # --- weighted message sum ----------------------------------
                opre = smallp.tile([128, WMAX * 128], F32, tag="opre")
                opret = smallp.tile([128, WMAX * 128], F32, tag="opret")
                goff = 0
                first = True
                for g in range(NGRP):
                    if Ug[g] == 0:
                        continue
                    Kg = W * Ug[g]
                    hv = stag[:, goff * row:(goff + Kg) * row] \
                        .rearrange("p (k r) -> p k r", r=row)[:, :, hoff:hoff + nh * nch]
                    msg = scrp.tile([128, MAXGRP], BF16, tag="pr")
                    wv_ = wall_t[:, goff * nh:(goff + Kg) * nh]
                    if layer == 1:
                        mv = msg[:, 0:Kg * nh * nch].rearrange(
                            "p (k c h) -> p k c h", c=nch, h=nh)
                        nc.vector.tensor_tensor(
                            out=mv,
                            in0=hv.rearrange("p k (c h) -> p k c h", c=nch),
                            in1=wv_.rearrange("p (k c h) -> p k c h",
                                              c=1, h=nh)
                                .to_broadcast([128, Kg, nch, nh]),
                            op=OP.mult)
                    else:
                        mv = msg[:, 0:Kg * nh * nch].rearrange(
                            "p (k h c) -> p k h c", h=nh, c=nch)
                        nc.vector.tensor_tensor(
                            out=mv,
                            in0=hv.rearrange("p k (h c) -> p k h c", h=nh),
                            in1=wv_.rearrange("p (k h c) -> p k h c",
                                              h=nh, c=1)
                                .to_broadcast([128, Kg, nh, nch]),
                            op=OP.mult)
                    # in-place tree reduction over slots l: contiguous
                    # innermost (e) keeps the DVE 16-bit fast path; pad
                    # slots contribute exact zeros.
                    E_ = nh * nch
                    L = Ug[g]
                    while L > 1:
                        h_ = L // 2
                        v3 = msg[:, 0:Kg * E_].rearrange(
                            "p (w l e) -> p w l e", w=W, l=Ug[g])
                        nc.vector.tensor_tensor(
                            out=v3[:, :, 0:h_, :], in0=v3[:, :, 0:h_, :],
                            in1=v3[:, :, L - h_:L, :], op=OP.add)
                        L -= h_
                    tgt = opre if first else opret
                    nc.vector.tensor_copy(
                        out=tgt[:, 0:W * E_].rearrange("p (w e) -> p w e", w=W),
                        in_=msg[:, 0:Kg * E_].rearrange(
                            "p (w l e) -> p w l e", w=W, l=Ug[g])[:, :, 0:1, :]
                            .rearrange("p w l e -> p w (l e)"))
                    if not first:
                        nc.vector.tensor_tensor(out=opre[:, 0:W * nh * nch],
                                                in0=opre[:, 0:W * nh * nch],
                                                in1=opret[:, 0:W * nh * nch],
                                                op=OP.add)
                    first = False
                    goff += Kg
                # --- normalize + bias --------------------------------------
                o1 = smallp.tile([128, WMAX * 128], F32, tag="o1")
                if layer == 1:
                    nc.vector.tensor_tensor(
                        out=o1[:, 0:W * nh * nch].rearrange(
                            "p (w c h) -> p w c h", c=nch, h=nh),
                        in0=opre[:, 0:W * nh * nch].rearrange(
                            "p (w c h) -> p w c h", c=nch, h=nh),
                        in1=rec[:, 0:W * nh].rearrange(
                            "p (w c h) -> p w c h", c=1, h=nh)
                            .to_broadcast([128, W, nch, nh]),
                        op=OP.mult)
                else:
                    nc.vector.tensor_tensor(
                        out=o1[:, 0:W * nh * nch].rearrange(
                            "p (w h c) -> p w h c", h=nh, c=nch),
                        in0=opre[:, 0:W * nh * nch].rearrange(
                            "p (w h c) -> p w h c", h=nh, c=nch),
                        in1=rec[:, 0:W * nh].rearrange(
                            "p (w h c) -> p w h c", h=nh, c=1)
                            .to_broadcast([128, W, nh, nch]),
                        op=OP.mult)
                bv = b1v if layer == 1 else b2v
                nc.vector.tensor_tensor(
                    out=o1[:, 0:W * nh * nch].rearrange(
                        "p (w e) -> p w e", w=W),
                    in0=o1[:, 0:W * nh * nch].rearrange(
                        "p (w e) -> p w e", w=W),
                    in1=bv.rearrange("p (w e) -> p w e", w=1)
                        .to_broadcast([128, W, nh * nch]),
                    op=OP.add)
                if layer == 1:
                    # ELU -> h2 rows -> shard2
                    ne = W * 128
                    tneg = smallp.tile([128, WMAX * 128], F32, tag="tneg")
                    nc.vector.tensor_scalar_min(tneg[:, 0:ne], o1[:, 0:ne], 0.0)
                    nc.scalar.activation(tneg[:, 0:ne], tneg[:, 0:ne],
                                         ACT.Exp, 0.0, 1.0)
                    nc.vector.tensor_scalar_max(o1[:, 0:ne], o1[:, 0:ne], 0.0)
                    nc.vector.tensor_tensor(out=o1[:, 0:ne], in0=o1[:, 0:ne],
                                            in1=tneg[:, 0:ne], op=OP.add)
                    nc.vector.tensor_scalar_add(o1[:, 0:ne], o1[:, 0:ne], -1.0)
                    o1bf = smallp.tile([128, WMAX * 128], BF16, tag="o1bf")
                    nc.vector.tensor_copy(out=o1bf[:, 0:ne], in_=o1[:, 0:ne])
                    ps2 = psump.tile([128, WMAX * 66], F32, tag="ps2")
                    for wi in range(W):
                        pst = psump.tile([128, 128], BF16, tag="pst")
                        nc.tensor.transpose(
                            out=pst[:], in_=o1bf[:, wi * 128:(wi + 1) * 128],
                            identity=ident[:])
                        o1T = smallp.tile([128, 128], BF16, tag="o1T")
                        nc.vector.tensor_copy(out=o1T[:], in_=pst[:])
                        nc.tensor.matmul(out=ps2[:, wi * 66:(wi + 1) * 66],
                                         lhsT=o1T[:], rhs=w2r_t[:],
                                         start=True, stop=True)
                    row2 = smallp.tile([128, WMAX * 65], BF16, tag="row2")
                    nc.vector.tensor_copy(
                        out=row2[:, 0:W * 65].rearrange(
                            "p (w r) -> p w r", w=W),
                        in_=ps2[:, 0:W * 66].rearrange(
                            "p (w r) -> p w r", w=W)[:, :, 0:65])
                    nc.vector.tensor_copy(
                        out=ad2win[:, w0:w0 + W].rearrange(
                            "p (w r) -> p w r", r=1),
                        in_=ps2[:, 0:W * 66].rearrange(
                            "p (w r) -> p w r", w=W)[:, :, 65:66])
                    nc.sync.dma_start(
                        out=shard2[w0 * 128:(w0 + W) * 128, 0:65]
                            .rearrange("(a p) r -> p a r", p=128),
                        in_=row2[:, 0:W * 65].rearrange("p (a r) -> p a r", a=W))
                else:
                    # log-softmax: sh and se into persistent buffers
                    ne = W * OUT
                    mx = smallp.tile([128, WMAX], F32, tag="mx")
                    nc.vector.tensor_reduce(
                        out=mx[:, 0:W].rearrange("p (w e) -> p w e", e=1),
                        in_=o1[:, 0:ne].rearrange("p (w c) -> p w c", w=W),
                        axis=AX.X, op=OP.max)
                    shv = shbuf[:, w0 * OUT:(w0 + W) * OUT]
                    nc.vector.tensor_tensor(
                        out=shv.rearrange("p (w c) -> p w c", w=W),
                        in0=o1[:, 0:ne].rearrange("p (w c) -> p w c", w=W),
                        in1=mx[:, 0:W].rearrange("p (w c) -> p w c", c=1)
                            .to_broadcast([128, W, OUT]),
                        op=OP.subtract)
                    ex = smallp.tile([128, WMAX * OUT], F32, tag="ex")
                    nc.scalar.activation(ex[:, 0:ne], shv, ACT.Exp, 0.0, 1.0)
                    nc.vector.tensor_reduce(
                        out=sebuf[:, w0:w0 + W].rearrange(
                            "p (w e) -> p w e", e=1),
                        in_=ex[:, 0:ne].rearrange("p (w c) -> p w c", w=W),
                        axis=AX.X, op=OP.add)

            def edge_layer(layer):
                nb = int(os.environ.get("GAT_NBATCH", "999"))
                todo = (batches1 if layer == 1 else batches2)[:nb]
                if not todo:
                    return
                pend = [(todo[0], *load_batch(layer, todo[0]))]
                for b in todo[1:]:
                    bp, st, gt = pend.pop(0)
                    # await first so the anchor isn't queued behind the next
                    # batch's descriptor generation on Pool; the prefetch then
                    # runs on Pool concurrently with this batch's DVE compute.
                    await_gather(layer, bp, st, gt)
                    pend.append((b, *load_batch(layer, b)))
                    compute_batch(layer, bp, st)
                bp, st, gt = pend.pop(0)
                await_gather(layer, bp, st, gt)
                compute_batch(layer, bp, st)

            STAGE = int(os.environ.get("GAT_STAGE", "3"))
            if STAGE >= 1:
                edge_layer(1)
            # pad row for table2: a_s2 = -300
            pr2 = constp.tile([1, 1], BF16, tag="pr2")
            nc.vector.memset(pr2[:], A_NEG)
            nc.sync.dma_start(out=shard2[PAD2:PAD2 + 1, 64:65], in_=pr2[0:1, :])
            if STAGE >= 2:
                with tc.tile_critical():
                    nc.gpsimd.collective_compute(
                        "AllGather", OP.bypass,
                        replica_groups=[list(range(NCORES))],
                        ins=[shard2[:]], outs=[table2[:]],
                    ).then_inc(cc_sem, 1)
                    nc.gpsimd.wait_ge(cc_sem, 2)
                cn[0] += 1
            if STAGE >= 3:
                edge_layer(2)
                # final: out = sh - ln(se), one DMA
                nc.scalar.activation(sebuf[:], sebuf[:], ACT.Ln, 0.0, 1.0)
                nc.vector.tensor_tensor(
                    out=shbuf[:].rearrange("p (w c) -> p w c", w=NW),
                    in0=shbuf[:].rearrange("p (w c) -> p w c", w=NW),
                    in1=sebuf[:].rearrange("p (w c) -> p w c", c=1)
                        .to_broadcast([128, NW, OUT]),
                    op=OP.subtract)
                nc.sync.dma_start(
                    out=outp[:].rearrange("(a p) r -> p a r", p=128),
                    in_=shbuf[:].rearrange("p (a r) -> p a r", a=NW))
            else:
                zo = smallp.tile([128, OUT], F32, tag="zo")
                nc.vector.memset(zo[:], 0.0)
                for w in range(NW):
                    nc.sync.dma_start(out=outp[w * 128:(w + 1) * 128, :],
                                      in_=zo[:])
            ctx_edge.close()
    nc.compile()
    return nc


_CACHE = {}


def kernel(**inputs):
    ei = np.asarray(inputs["edge_index"])
    src, dst = ei[0].astype(np.int64), ei[1].astype(np.int64)
    lay = _layout(src, dst)
    batches1 = _make_batches(lay["Lg"], COLS1)
    batches2 = _make_batches(lay["Lg"], COLS2)
    per_core = _host_inputs(inputs, lay, batches1, batches2)
    key = (ei.tobytes()[:64], int(lay["Lg"].sum()))
    if key not in _CACHE:
        _CACHE[key] = _build_program(lay["Lg"], batches1, batches2)
    nc = _CACHE[key]
    res = run_bass_kernel_spmd(nc, per_core, core_ids=list(range(NCORES)))
    out = np.empty((N, OUT), np.float32)
    for k in range(NCORES):
        out[k * NSH + lay["perms"][k]] = res.results[k]["out"][:NSH]
    return out


if __name__ == "__main__":
    d = np.load("/root/problem/_inp_check.npz")
    o = kernel(**{k: d[k] for k in d.files})
    ref = np.load("/root/problem/_ref_check.npy")
    rel = np.linalg.norm(o - ref) / np.linalg.norm(ref)
    err = np.abs(o - ref) / (np.abs(ref) + 1e-5)
    print("fro rel err:", rel, "max elem rel err:", err.max())
